# revision 62
# baseline (speedup 1.0000x reference)
"""CLD sde_reverse (Riemann geometry) Trainium2 kernel — v5.

Contract: kernel(u, score_x, t) -> (drift, diffusion), full (unsharded) numpy
arrays, computed on 8 NeuronCores via bass/Tile + run_bass_kernel_spmd.

Sharding: pixels (image rows) are sharded 8 ways; every core sees all 64 batch
elements for its 32 rows. The batch-mean outer product G, the 3x3 per-pixel
algebra, and the drift matmuls are all per-pixel, so there are no cross-core
dependencies and no collectives.

Math (per pixel, 3x3):
    G     = alpha * (mean_b s s^T) + c*I,   c = (1-alpha)/m_inv
    L     = chol(G),  A = beta * L @ inv(G)
    drift_x = A @ r
    drift_r = -(beta*L @ x + beta*Gamma * r)     (G @ inv(G) = I exactly)
    diffusion_x = 0
    diffusion_r = sqrt(2*beta*Gamma) * (L @ 1)   (batch independent)

Taylor mode (used when alpha/c is small, as it is for the reference t):
    G = c (I + E),  E = (alpha/(B c)) * S,  S = sum_b s s^T,  |E| ~ alpha/c
    L = sqrt(c) (I + tril(E) + diag(E)/2)  + O(E^2)
    A = (beta/sqrt(c)) (I - diag(E)/2 - triu_strict(E))  + O(E^2)
  so every drift coefficient is a single affine map of one entry of S —
  it drains straight out of the pair-reduction PSUM with the scale/bias
  folded into the (exact, non-table) AF.Copy activation.  No cholesky, no
  inverse, and drift_x's lower triangle vanishes (6 products instead of 9).
  The O(E^2) truncation error is ~(alpha/c)^2 * ghat^2 ~ 1e-3 relative at
  the reference t0; kernel() falls back to the exact path if alpha/c is
  not small.

Engine split: pair planes s_i*s_j on ACT (squares) / DVE (cross mults),
fold 32->16 on DVE (fp16 adds are fast), 16 b-slice identity matmuls per
half on the PE accumulate the batch sum in PSUM.  Main stage: coefficient-
broadcast products on DVE (a few on Pool), identity-matmul accumulation on
the PE (drift_r's -beta*Gamma*r enters via a BG-scaled identity straight
from the r tile), one ACT/DVE drain per (channel, batch-half), streaming
output DMAs.
"""

import math

import numpy as np

# ---- model constants (from the reference config) ----
M_INV = 4.0
GAMMA_BIG = 0.04
BETA0 = 4.0
RIEMANN_MIX = 0.5
K_DECAY = 4.5
C = 3
HW = 256
B = 64

N_CORES = 8
ROWS = HW // N_CORES  # 32 rows per core
P = 128               # SBUF partitions
PL = (ROWS * HW) // P  # 64 free pixels per partition

BETA_C = BETA0 * math.sqrt(M_INV)        # 8.0
GAMMA_C = GAMMA_BIG * math.sqrt(M_INV)   # 0.08
BG = BETA_C * GAMMA_C                    # 0.64
BG_SCALE = math.sqrt(2.0 * BETA_C * GAMMA_C)

TAYLOR_MAX_RATIO = 0.03   # use the Taylor path when alpha/c <= this

_PROG_CACHE: dict = {}


def _build_program(ca: float, cid: float, n_reps: int = 1,
                   taylor: bool = True, newton: bool = True,
                   pool_products: int = 2, pool_pairs: bool = True,
                   pool_dif: bool = False, serialize: bool = False):
    """Build + compile the per-core SPMD bass program.

    ca  = alpha / B     (scale for the raw sum S_ij; normalization == 1)
    cid = (1 - alpha) / M_INV
    """
    from contextlib import ExitStack

    import concourse.bacc as bacc
    import concourse.mybir as mybir
    import concourse.tile as tile

    dt = mybir.dt
    op = mybir.AluOpType
    f32 = dt.float32
    f16 = dt.float16
    AF = mybir.ActivationFunctionType

    nc = bacc.Bacc("TRN2", target_bir_lowering=False, debug=False,
                   num_devices=N_CORES)

    s_in = nc.dram_tensor("s_in", [C, P, B, PL], f16,
                          kind="ExternalInput").ap()
    u_in = nc.dram_tensor("u_in", [2 * C, P, B, PL], f16,
                          kind="ExternalInput").ap()
    id_in = nc.dram_tensor("ident", [P, 2 * P], f16,
                           kind="ExternalInput").ap()
    drift_o = nc.dram_tensor("drift", [2 * C, P, B, PL], f16,
                             kind="ExternalOutput").ap()
    dif_o = nc.dram_tensor("dif", [C, P, PL], f32, kind="ExternalOutput").ap()

    HB = B // 2   # batch half
    sqc = math.sqrt(cid)
    k_e = ca / cid           # PSUM pair-sum -> E entry
    bL = BETA_C * sqc        # beta * sqrt(c)
    bA = BETA_C / sqc        # beta / sqrt(c)

    with tile.TileContext(nc) as tc:
      for _rep in range(n_reps):
        with ExitStack() as stack:
            coef = stack.enter_context(tc.tile_pool(name="coef", bufs=1))
            data = stack.enter_context(tc.tile_pool(name="data", bufs=1))
            tmp = stack.enter_context(tc.tile_pool(name="tmp", bufs=2))
            score_pool = stack.enter_context(
                tc.tile_pool(name="score", bufs=1))
            prod_pool = stack.enter_context(tc.tile_pool(name="prod",
                                                         bufs=1))
            psum_low = stack.enter_context(
                tc.tile_pool(name="psum_low", bufs=1, space="PSUM"))
            psum = stack.enter_context(
                tc.tile_pool(name="psum", bufs=3, space="PSUM"))

            # pin the sqrt-containing ACT table before any Square runs
            # (stray table swaps cost 1.3us mid-kernel)
            dum = tmp.tile([P, 1], f32, tag="dum")
            nc.scalar.memzero(dum[:])
            nc.scalar.activation(dum[:], dum[:], AF.Sqrt)

            ident2 = coef.tile([P, 2 * P], f16, tag="ident2")
            # off the SP ring so the score DMAs dispatch immediately
            nc.scalar.dma_start(out=ident2[:], in_=id_in)
            ident = ident2[:, 0:P]
            bgident = ident2[:, P:2 * P]

            # sacrificial first PE accumulation group: the program's first
            # matmul group does not land reliably on HW, so burn one into
            # a scratch bank before any real accumulation
            low_guard = psum_low.tile([P, 512], f32, tag="low_guard")
            nc.tensor.matmul(low_guard[:, 0:P], ident, ident2[:, 0:P],
                             start=True, stop=False, skip_group_check=True)
            nc.tensor.matmul(low_guard[:, 0:P], ident, ident2[:, 0:P],
                             start=False, stop=True, skip_group_check=True)

            # ---------------- input DMA streams ---------------------------
            # score: s0/s1 halves first (their pairs gate the L start)
            s_t = [score_pool.tile([P, B, PL], f16, tag=f"s{c}",
                                   name=f"s{c}") for c in range(C)]
            for (c, bh) in [(0, 0), (1, 0), (0, 1), (1, 1), (2, 0), (2, 1)]:
                bsl = slice(bh * HB, (bh + 1) * HB)
                nc.sync.dma_start(out=s_t[c][:, bsl, :],
                                  in_=s_in[c, :, bsl, :])
            # u in wave consumption order: drift_r group i of half bh needs
            # x_0..i[bh] for products and r_i[bh] for its BG matmul
            u_t = [data.tile([P, B, PL], f16, tag=f"u{c}",
                             name=f"u{c}") for c in range(2 * C)]
            for bh in range(2):
                bsl = slice(bh * HB, (bh + 1) * HB)
                for c in (0, C + 0, 1, C + 1, 2, C + 2):
                    nc.sync.dma_start(out=u_t[c][:, bsl, :],
                                      in_=u_in[c, :, bsl, :])
            x_t, r_t = u_t[:C], u_t[C:]

            # ---------------- stage A machinery ----------------------------
            pairs_all = [(0, 0), (0, 1), (1, 1), (0, 2), (1, 2), (2, 2)]
            # all six pair accumulators packed into one PSUM bank

            half_sums = {}
            pair_R = {}

            def pair_half(i, j, bh, pool=False):
                # plane + full fp16 fold tree to [P,1,PL] on DVE; halves
                # combine into an f32 per-pair sum (no PE involvement:
                # interleaved PE accumulation groups misbehave on HW)
                bsl = slice(bh * HB, (bh + 1) * HB)
                ph = prod_pool.tile([P, HB, PL], f16, tag="ph",
                                    bufs=6, name="ph")
                if i == j:
                    nc.scalar.activation(ph[:], s_t[i][:, bsl, :],
                                         AF.Square)
                else:
                    eng = nc.gpsimd if pool else nc.vector
                    eng.tensor_tensor(
                        ph[:], s_t[i][:, bsl, :], s_t[j][:, bsl, :],
                        op.mult)
                cur, n = ph, HB
                while n > 1:
                    h = n // 2
                    nxt = prod_pool.tile([P, h, PL], f16, tag=f"fold{h}",
                                         bufs=4, name="fold")
                    nc.vector.tensor_tensor(nxt[:], cur[:, 0:h, :],
                                            cur[:, h:n, :], op.add)
                    cur, n = nxt, h
                if bh == 0:
                    half_sums[(i, j)] = cur
                else:
                    R = coef.tile([P, 1, PL], f32, tag=f"R{i}{j}",
                                  name=f"R{i}{j}")
                    nc.vector.tensor_tensor(R[:], half_sums[(i, j)][:],
                                            cur[:], op.add)
                    pair_R[(i, j)] = R

            def cdrain(pp, scale, bias, tag):
                # coefficient = scale * pair_sum + bias (AF.Copy: exact
                # datapath, no table)
                e = coef.tile([P, 1, PL], f16, tag=tag, name=tag)
                nc.scalar.activation(e[:, 0, :], pair_R[pp][:, 0, :],
                                     AF.Copy, bias=float(bias),
                                     scale=float(scale))
                return e

            # ---------------- stage C machinery ----------------------------
            mtmp = stack.enter_context(tc.tile_pool(name="mtmp", bufs=2))
            outs = stack.enter_context(tc.tile_pool(name="outs", bufs=3))

            def emit(ch_i, bh, coeffs, ins, with_bg, pool_idx=None,
                     drain_dve=False):
                bsl = slice(bh * HB, (bh + 1) * HB)
                n_pe = len(coeffs) + (1 if with_bg else 0)
                prs = []
                for idx, (cc, dd) in enumerate(zip(coeffs, ins)):
                    pr = mtmp.tile([P, HB, PL], f16, tag=f"pr{idx}", bufs=3,
                                   name=f"pr{idx}")
                    bc = cc.broadcast_to([P, HB, PL])
                    eng = nc.gpsimd if idx == pool_idx else nc.vector
                    eng.tensor_tensor(pr[:], dd[:, bsl, :], bc, op.mult)
                    prs.append(pr)
                pss = [psum.tile([P, 1024], f32, tag="ps", bufs=3,
                                 name="ps") for _ in range(2)]
                # term-major, products first (the BG term last: its r half
                # may still be streaming in when the group starts)
                idx = 0
                for pr in prs:
                    rhs = pr[:].rearrange("p b l -> p (b l)")
                    for s2 in range(4):
                        sl = slice(s2 * 512, (s2 + 1) * 512)
                        psl = slice((s2 % 2) * 512, (s2 % 2 + 1) * 512)
                        nc.tensor.matmul(
                            pss[s2 // 2][:, psl], ident, rhs[:, sl],
                            start=(idx == 0), stop=(idx == n_pe - 1),
                            skip_group_check=True)
                    idx += 1
                if with_bg:
                    rfull = r_t[ch_i][:].rearrange("p b l -> p (b l)")
                    for s2 in range(4):
                        sl = slice((s2 % 2) * 512, (s2 % 2 + 1) * 512)
                        gl = slice(bh * 2048 + s2 * 512,
                                   bh * 2048 + (s2 + 1) * 512)
                        nc.tensor.matmul(
                            pss[s2 // 2][:, sl], bgident, rfull[:, gl],
                            start=(idx == 0), stop=True,
                            skip_group_check=True)
                    idx += 1
                sign = -1.0 if with_bg else 1.0
                out_ch = (C + ch_i) if with_bg else ch_i
                o = outs.tile([P, HB, PL], f16, tag="o", name="o")
                for hq, ps in enumerate(pss):
                    src = ps[:].rearrange("p (b l) -> p b l", b=HB // 2)
                    dst = o[:, hq * (HB // 2):(hq + 1) * (HB // 2), :]
                    if drain_dve:
                        nc.vector.tensor_scalar(dst, src, sign, None,
                                                op.mult)
                    else:
                        nc.scalar.mul(dst, src, sign)
                nc.sync.dma_start(out=drift_o[out_ch, :, bsl, :], in_=o[:])
                return o

            # ---------------- the schedule ---------------------------------
            for (i, j) in [(0, 0), (0, 1), (1, 1)]:
                pair_half(i, j, 0)
            for (i, j) in [(0, 0), (0, 1), (1, 1)]:
                pair_half(i, j, 1)

            if taylor:
                eL, eA, o_last = _taylor_body(
                    nc, tc, mybir, coef, tmp, pair_half, cdrain, emit,
                    k_e, bL, bA, x_t, r_t, pool_products, pool_pairs,
                    pool_dif, dif_o)
            else:
                eL, eA, o_last = _exact_body(
                    nc, tc, mybir, coef, tmp, pair_half, pair_R, emit,
                    float(ca), float(cid), newton, x_t, r_t,
                    pool_products, dif_o)

            if serialize and _rep < n_reps - 1:
                # measurement mode: chain rep boundaries so the per-rep
                # slope equals the single-pass span
                tok = tmp.tile([P, 1], f16, tag="tok")
                nc.vector.tensor_tensor(tok[:], s_t[0][:, 0, 0:1],
                                        o_last[:, 0, 0:1], op.add)
                tok2 = tmp.tile([P, 1], f16, tag="tok2")
                nc.vector.tensor_tensor(tok2[:], u_t[0][:, 0, 0:1],
                                        o_last[:, 0, 0:1], op.add)

    nc.compile()
    return nc


def _taylor_body(nc, tc, mybir, coef, tmp, pair_half, cdrain, emit,
                 k_e, bL, bA, x_t, r_t, pool_products, pool_pairs,
                 pool_dif, dif_o):
    """First-order coefficient path: every coefficient is one PSUM drain."""
    op = mybir.AluOpType
    f32 = mybir.dt.float32

    eLt = {}
    eAt = {}
    # s0/s1 coefficient drains -> first drift_r groups
    eLt[(0, 0)] = cdrain((0, 0), k_e * bL / 2, bL, "eL00")
    eLt[(1, 0)] = cdrain((0, 1), k_e * bL, 0.0, "eL10")
    eLt[(1, 1)] = cdrain((1, 1), k_e * bL / 2, bL, "eL11")

    def ap(d, i, j):
        return d[(i, j)][:]

    emit(0, 0, [ap(eLt, 0, 0)], x_t[:1], True)
    emit(1, 0, [ap(eLt, 1, 0), ap(eLt, 1, 1)], x_t[:2], True)

    # s2 pairs ((2,2) squares first on ACT; (1,2) optionally on Pool)
    pair_half(2, 2, 0)
    pair_half(2, 2, 1)
    pair_half(0, 2, 0)
    pair_half(1, 2, 0, pool=pool_pairs)
    pair_half(0, 2, 1)
    pair_half(1, 2, 1, pool=pool_pairs)
    eLt[(2, 0)] = cdrain((0, 2), k_e * bL, 0.0, "eL20")
    eLt[(2, 1)] = cdrain((1, 2), k_e * bL, 0.0, "eL21")
    eLt[(2, 2)] = cdrain((2, 2), k_e * bL / 2, bL, "eL22")
    emit(2, 0, [ap(eLt, 2, j) for j in range(3)], x_t, True)
    for i in range(3):
        emit(i, 1, [ap(eLt, i, j) for j in range(i + 1)], x_t[:i + 1],
             True)

    # drift_x coefficients (upper triangle only at first order)
    eAt[(0, 0)] = cdrain((0, 0), -k_e * bA / 2, bA, "eA00")
    eAt[(0, 1)] = cdrain((0, 1), -k_e * bA, 0.0, "eA01")
    eAt[(0, 2)] = cdrain((0, 2), -k_e * bA, 0.0, "eA02")
    eAt[(1, 1)] = cdrain((1, 1), -k_e * bA / 2, bA, "eA11")
    eAt[(1, 2)] = cdrain((1, 2), -k_e * bA, 0.0, "eA12")
    eAt[(2, 2)] = cdrain((2, 2), -k_e * bA / 2, bA, "eA22")

    # diffusion_r rows = (BG_SCALE/beta) * row sums of (beta*L)
    kd = BG_SCALE / BETA_C
    deng = nc.gpsimd if pool_dif else nc.vector

    dif0 = coef.tile([P, PL], f32, tag="dif0")
    deng.tensor_scalar(dif0[:], eLt[(0, 0)][:, 0, :], kd, None, op.mult)
    t1 = coef.tile([P, PL], f32, tag="difs1")
    deng.tensor_tensor(t1[:], eLt[(1, 0)][:, 0, :],
                       eLt[(1, 1)][:, 0, :], op.add)
    dif1 = coef.tile([P, PL], f32, tag="dif1")
    deng.tensor_scalar(dif1[:], t1[:], kd, None, op.mult)
    t2 = coef.tile([P, PL], f32, tag="difs2a")
    deng.tensor_tensor(t2[:], eLt[(2, 0)][:, 0, :],
                       eLt[(2, 1)][:, 0, :], op.add)
    t3 = coef.tile([P, PL], f32, tag="difs2")
    deng.tensor_tensor(t3[:], t2[:], eLt[(2, 2)][:, 0, :], op.add)
    dif2 = coef.tile([P, PL], f32, tag="dif2")
    deng.tensor_scalar(dif2[:], t3[:], kd, None, op.mult)
    for i, dtile in enumerate((dif0, dif1, dif2)):
        nc.sync.dma_start(out=dif_o[i], in_=dtile[:])

    # drift_x waves: row i needs only channels j >= i
    o_last = None
    npool = 0
    for bh in range(2):
        for i in range(3):
            coeffs = [ap(eAt, i, j) for j in range(i, 3)]
            ins = r_t[i:]
            pidx = None
            if bh == 1 and len(coeffs) >= 2 and npool < pool_products:
                pidx = 0
                npool += 1
            o_last = emit(i, bh, coeffs, ins, False, pool_idx=pidx,
                          drain_dve=(bh == 1 and i == 2))
    return eLt, eAt, o_last


def _exact_body(nc, tc, mybir, coef, tmp, pair_half, pair_R, emit,
                ca, cid, newton, x_t, r_t, pool_products, dif_o):
    """Exact cholesky/adjugate path (fallback for large alpha/c)."""
    op = mybir.AluOpType
    f32 = mybir.dt.float32
    f16 = mybir.dt.float16
    AF = mybir.ActivationFunctionType

    g = {}

    def gdrain(i, j):
        gij = coef.tile([P, PL], f32, tag=f"g{i}{j}", name=f"g{i}{j}")
        bias = float(cid) if i == j else 0.0
        nc.scalar.activation(
            gij[:], pair_R[(i, j) if i <= j else (j, i)][:, 0, :],
            AF.Copy, bias=bias, scale=float(ca))
        g[(i, j)] = gij
        g[(j, i)] = gij

    def sqrt_ref(a, tag):
        out = coef.tile([P, PL], f32, tag=tag, name=tag)
        nc.scalar.activation(out[:], a[:], AF.Sqrt)
        if not newton:
            return out
        r0 = tmp.tile([P, PL], f32, tag="sqr")
        nc.vector.reciprocal(r0[:], out[:])
        ar = tmp.tile([P, PL], f32, tag="sqar")
        nc.vector.tensor_tensor(ar[:], a[:], r0[:], op.mult)
        ref = coef.tile([P, PL], f32, tag=tag + "n", name=tag + "n")
        nc.vector.tensor_tensor(ref[:], out[:], ar[:], op.add)
        out2 = coef.tile([P, PL], f32, tag=tag + "h", name=tag + "h")
        nc.vector.tensor_scalar(out2[:], ref[:], 0.5, None, op.mult)
        return out2

    def tt(a, b_, o, tag):
        t = coef.tile([P, PL], f32, tag=tag, name=tag)
        nc.vector.tensor_tensor(t[:], a[:], b_[:], o)
        return t

    def ecopy(lt, i, j):
        e = coef.tile([P, 1, PL], f16, tag=f"eL{i}{j}", name=f"eL{i}{j}")
        nc.scalar.mul(e[:, 0, :], lt[:], BETA_C)
        return e[:]

    gdrain(0, 0)
    l00 = sqrt_ref(g[0, 0], "l00")
    gdrain(0, 1)
    gdrain(1, 1)
    eL = {(0, 0): ecopy(l00, 0, 0)}
    il00 = coef.tile([P, PL], f32, tag="il00")
    nc.vector.reciprocal(il00[:], l00[:])
    l10 = tt(g[0, 1], il00, op.mult, "l10")
    emit(0, 0, [eL[(0, 0)]], x_t[:1], True)
    t = tt(l10, l10, op.mult, "l10sq")
    dd1 = tt(g[1, 1], t, op.subtract, "dd1")
    l11 = sqrt_ref(dd1, "l11")
    il11 = coef.tile([P, PL], f32, tag="il11")
    nc.vector.reciprocal(il11[:], l11[:])
    eL[(1, 0)] = ecopy(l10, 1, 0)
    eL[(1, 1)] = ecopy(l11, 1, 1)
    emit(1, 0, [eL[(1, 0)], eL[(1, 1)]], x_t[:2], True)

    pair_half(2, 2, 0)
    pair_half(2, 2, 1)
    pair_half(0, 2, 0)
    pair_half(1, 2, 0)
    pair_half(0, 2, 1)
    pair_half(1, 2, 1)
    gdrain(0, 2)
    gdrain(1, 2)
    gdrain(2, 2)
    l20 = tt(g[0, 2], il00, op.mult, "l20")
    t = tt(l20, l10, op.mult, "l20l10")
    t = tt(g[1, 2], t, op.subtract, "g12m")
    l21 = tt(t, il11, op.mult, "l21")
    t = tt(l20, l20, op.mult, "l20sq")
    dd2 = tt(g[2, 2], t, op.subtract, "dd2a")
    t = tt(l21, l21, op.mult, "l21sq")
    dd2 = tt(dd2, t, op.subtract, "dd2")
    l22 = sqrt_ref(dd2, "l22")
    L = {(0, 0): l00, (1, 0): l10, (1, 1): l11,
         (2, 0): l20, (2, 1): l21, (2, 2): l22}
    eL[(2, 0)] = ecopy(l20, 2, 0)
    eL[(2, 1)] = ecopy(l21, 2, 1)
    eL[(2, 2)] = ecopy(l22, 2, 2)
    emit(2, 0, [eL[(2, j)] for j in range(3)], x_t, True)
    for i in range(3):
        emit(i, 1, [eL[(i, j)] for j in range(i + 1)], x_t[:i + 1], True)

    # adjugate + det on Pool
    def ptt(a, b_, o, tag):
        t = coef.tile([P, PL], f32, tag=tag, name=tag)
        nc.gpsimd.tensor_tensor(t[:], a[:], b_[:], o)
        return t

    def fmsub(a, b_, c_, d_, tag):
        t1 = tmp.tile([P, PL], f32, tag="fm1")
        nc.gpsimd.tensor_tensor(t1[:], a[:], b_[:], op.mult)
        t2 = tmp.tile([P, PL], f32, tag="fm2")
        nc.gpsimd.tensor_tensor(t2[:], c_[:], d_[:], op.mult)
        t_ = coef.tile([P, PL], f32, tag=tag, name=tag)
        nc.gpsimd.tensor_tensor(t_[:], t1[:], t2[:], op.subtract)
        return t_

    c00 = fmsub(g[1, 1], g[2, 2], g[1, 2], g[1, 2], "c00")
    c01 = fmsub(g[0, 2], g[1, 2], g[0, 1], g[2, 2], "c01")
    c02 = fmsub(g[0, 1], g[1, 2], g[0, 2], g[1, 1], "c02")
    c11 = fmsub(g[0, 0], g[2, 2], g[0, 2], g[0, 2], "c11")
    c12 = fmsub(g[0, 1], g[0, 2], g[0, 0], g[1, 2], "c12")
    c22 = fmsub(g[0, 0], g[1, 1], g[0, 1], g[0, 1], "c22")
    d0 = ptt(g[0, 0], c00, op.mult, "d0")
    d1 = ptt(g[0, 1], c01, op.mult, "d1")
    d2 = ptt(g[0, 2], c02, op.mult, "d2")
    det = ptt(d0, d1, op.add, "deta")
    det = ptt(det, d2, op.add, "det")
    rdet = coef.tile([P, PL], f32, tag="rdet")
    nc.vector.reciprocal(rdet[:], det[:])

    kd = BG_SCALE / BETA_C

    def pts(src, tag):
        d = coef.tile([P, PL], f32, tag=tag, name=tag)
        nc.gpsimd.tensor_scalar(d[:], src[:], BG_SCALE, None, op.mult)
        return d

    dif0 = pts(L[0, 0], "dif0")
    t = ptt(L[1, 0], L[1, 1], op.add, "difs1")
    dif1 = pts(t, "dif1")
    t = ptt(L[2, 0], L[2, 1], op.add, "difs2a")
    t = ptt(t, L[2, 2], op.add, "difs2")
    dif2 = pts(t, "dif2")
    for i, dtile in enumerate((dif0, dif1, dif2)):
        nc.sync.dma_start(out=dif_o[i], in_=dtile[:])

    IV = [coef.tile([P, 3, PL], f32, tag=f"IV{k}", name=f"IV{k}")
          for k in range(3)]
    for (i, j), cof in [((0, 0), c00), ((0, 1), c01), ((0, 2), c02),
                        ((1, 1), c11), ((1, 2), c12), ((2, 2), c22)]:
        nc.gpsimd.tensor_tensor(IV[i][:, j, :], cof[:], rdet[:], op.mult)
        if i != j:
            nc.gpsimd.tensor_tensor(IV[j][:, i, :], cof[:], rdet[:],
                                    op.mult)

    def lb(i, k):
        return L[(i, k)][:].rearrange(
            "p l -> p () l").broadcast_to([P, 3, PL])

    eA = {}
    for i in range(3):
        if i == 0:
            ar = coef.tile([P, 3, PL], f32, tag="AR0", name="AR0")
            nc.vector.tensor_tensor(ar[:], IV[0][:], lb(0, 0), op.mult)
        else:
            acc = tmp.tile([P, 3, PL], f32, tag="Aacc")
            nc.vector.tensor_tensor(acc[:], IV[0][:], lb(i, 0), op.mult)
            for k in range(1, i + 1):
                pr = tmp.tile([P, 3, PL], f32, tag="Apr")
                nc.vector.tensor_tensor(pr[:], IV[k][:], lb(i, k), op.mult)
                dst = (coef.tile([P, 3, PL], f32, tag=f"AR{i}",
                                 name=f"AR{i}")
                       if k == i else tmp.tile([P, 3, PL], f32, tag="Aacc"))
                nc.vector.tensor_tensor(dst[:], acc[:], pr[:], op.add)
                acc = dst
            ar = acc
        e = coef.tile([P, 3, PL], f16, tag=f"eAR{i}", name=f"eAR{i}")
        nc.vector.tensor_scalar(e[:], ar[:], BETA_C, None, op.mult)
        for j in range(3):
            eA[(i, j)] = e[:, j:j + 1, :]

    o_last = None
    npool = 0
    for bh in range(2):
        for i in range(3):
            pidx = None
            if bh == 1 and npool < pool_products:
                pidx = 0
                npool += 1
            o_last = emit(i, bh, [eA[(i, j)] for j in range(3)], r_t,
                          False, pool_idx=pidx,
                          drain_dve=(bh == 1 and i == 2))
    return eL, eA, o_last


def _host_reference(u, score_x, t):
    """Pure-numpy fallback (exact reference math); used only when t[0]==1.0
    (the stateful normalization branch, never hit with uniform t)."""
    x, r = u[:, :C], u[:, C:]
    s = np.transpose(score_x, (0, 2, 3, 1)).astype(np.float32)
    G = np.einsum("bhwi,bhwj->hwij", s, s) / np.float32(score_x.shape[0])
    t0 = t[0]
    diag_mean = np.mean(np.trace(G, axis1=-2, axis2=-1)) / C
    normalization = np.where(t0 == 1.0, diag_mean * M_INV, 1.0)
    G = G / normalization
    G_id = (1.0 / M_INV) * np.eye(C, dtype=G.dtype)
    alpha = RIEMANN_MIX * np.exp(-K_DECAY * (1.0 - t0))
    G = alpha * G + (1.0 - alpha) * G_id
    G_inv = np.linalg.inv(G).astype(np.float32)
    G_sqrt = np.linalg.cholesky(G).astype(np.float32)

    def mm(Amat, Bf):
        return np.einsum("hwij,bjhw->bihw", Amat, Bf).astype(np.float32)

    hr = mm(G_inv, r)
    drift_x = BETA_C * mm(G_sqrt, hr)
    drift_r = (-BETA_C * mm(G_sqrt, x) - BETA_C * GAMMA_C * mm(G, hr))
    diffusion_x = np.zeros_like(x)
    diffusion_r = BG_SCALE * mm(G_sqrt, np.ones_like(r))
    drift = np.concatenate((drift_x, drift_r), axis=1)
    diffusion = np.concatenate((diffusion_x, diffusion_r), axis=1)
    return drift, diffusion


def _make_in_maps(u, score_x):
    ident2 = np.zeros((P, 2 * P), dtype=np.float16)
    ident2[:, 0:P] = np.eye(P, dtype=np.float16)
    ident2[:, P:2 * P] = np.float16(BG) * np.eye(P, dtype=np.float16)
    in_maps = []
    for k in range(N_CORES):
        rows = slice(k * ROWS, (k + 1) * ROWS)
        s_np = (score_x[:, :, rows, :]
                .reshape(B, C, P, PL).transpose(1, 2, 0, 3)
                .astype(np.float16))
        u_np = (u[:, :, rows, :]
                .reshape(B, 2 * C, P, PL).transpose(1, 2, 0, 3)
                .astype(np.float16))
        in_maps.append({
            "s_in": np.ascontiguousarray(s_np),
            "u_in": np.ascontiguousarray(u_np),
            "ident": ident2,
        })
    return in_maps


def kernel(u, score_x, t):
    from concourse.bass_utils import run_bass_kernel_spmd

    u = np.ascontiguousarray(np.asarray(u, dtype=np.float32))
    score_x = np.ascontiguousarray(np.asarray(score_x, dtype=np.float32))
    t = np.asarray(t, dtype=np.float32)

    t0 = float(t[0])
    if t0 == 1.0:
        return _host_reference(u, score_x, t)

    alpha = RIEMANN_MIX * math.exp(-K_DECAY * (1.0 - t0))
    ca = alpha / B          # normalization == 1.0 on this branch
    cid = (1.0 - alpha) / M_INV
    taylor = (alpha / cid) <= TAYLOR_MAX_RATIO

    key = (round(ca, 12), round(cid, 12), taylor)
    nc = _PROG_CACHE.get(key)
    if nc is None:
        nc = _build_program(ca, cid, taylor=taylor)
        _PROG_CACHE[key] = nc

    in_maps = _make_in_maps(u, score_x)
    res = run_bass_kernel_spmd(nc, in_maps, list(range(N_CORES)))

    drift = np.empty((B, 2 * C, HW, HW), dtype=np.float32)
    diffusion = np.zeros((B, 2 * C, HW, HW), dtype=np.float32)
    for k in range(N_CORES):
        rows = slice(k * ROWS, (k + 1) * ROWS)
        dk = res.results[k]["drift"].astype(np.float32)     # [6, P, B, PL]
        drift[:, :, rows, :] = dk.transpose(2, 0, 1, 3).reshape(
            B, 2 * C, ROWS, HW)
        difk = res.results[k]["dif"].reshape(C, ROWS, HW)   # [3, P, PL]
        diffusion[:, C:, rows, :] = difk[None, :, :, :]
    return drift, diffusion
